# revision 1
# baseline (speedup 1.0000x reference)
"""Trainium2 Bass kernel for nn_LoRALinear (quantized linear + LoRA).

reference:
    w_dq = quant_dequant_int8_per_row(weight)          # [out, in]
    out  = x @ w_dq.T + (alpha/r) * (x @ la) @ lb      # [T, out]

Math identity used here:
    out = x @ (w_dq.T + 2.0 * (la @ lb)) = x @ W_eff

The quant-dequant + LoRA fold is cheap elementwise/skinny-matmul host prep;
the 550-GFLOP dense matmul runs on 8 NeuronCores, data-parallel over tokens.

Device kernel (per core, SPMD identical program):
    xt  [4096, 2048] bf16  - token shard, pre-transposed to [K, M]
    w   [4096, 4096] bf16  - W_eff, replicated
    out [2048, 4096] f32
W_eff's integer-quantized part is bf16-rounded; x is bf16-rounded; matmul
accumulates in fp32 PSUM -> ~2-4e-3 relative error vs the f32 reference.
"""

import numpy as np
import ml_dtypes

TOKENS, IN_F, OUT_F, R = 16384, 4096, 4096, 16
N_CORES = 8
TPC = TOKENS // N_CORES  # tokens per core: 2048
SCALING = 2.0  # alpha / r
P = 128
NS = 512  # out_feature stripe (one PSUM bank of f32)

_NC_CACHE = {}


def _build_nc(tpc=TPC, in_f=IN_F, out_f=OUT_F, ns=NS, repeat=1):
    import concourse.mybir as mybir
    import concourse.tile as tile
    from concourse import bacc

    nc = bacc.Bacc("TRN2", target_bir_lowering=False)

    xt = nc.dram_tensor("xt", [in_f, tpc], mybir.dt.bfloat16, kind="ExternalInput")
    w = nc.dram_tensor("w", [in_f, out_f], mybir.dt.bfloat16, kind="ExternalInput")
    out = nc.dram_tensor("out", [tpc, out_f], mybir.dt.float32, kind="ExternalOutput")

    ko_n = in_f // P   # k-outer tiles (32)
    mt_n = tpc // P    # token tiles (16)
    nt_n = out_f // ns  # out_f stripes (8)
    xc = min(2 * P, tpc)  # x fill chunk: 256 tokens (512B DMA lines)
    x_chunks = tpc // xc

    with tile.TileContext(nc) as tc:
        with (
            tc.tile_pool(name="xpool", bufs=1) as xpool,
            tc.tile_pool(name="wpool", bufs=2) as wpool,
            tc.tile_pool(name="opool", bufs=4) as opool,
            tc.tile_pool(name="pspool", bufs=4, space="PSUM") as pspool,
        ):
            # Whole x shard stays resident in SBUF (bf16: 128 KiB/partition).
            x_sb = xpool.tile([P, ko_n, tpc], mybir.dt.bfloat16)
            xt_r = xt.rearrange("(ko p) m -> p ko m", p=P)
            w_r = w.rearrange("(ko p) n -> p ko n", p=P)

            # Stripe 0 of W and the x chunks are interleaved so the first
            # psum group's matmuls wait only on the first chunks, not the
            # whole 21 MB: PE starts ~10us in, DMA streams under compute.
            kc_n = 4  # stripe-0 ko chunks
            kcs = ko_n // kc_n
            w_sb0 = wpool.tile([P, ko_n, ns], mybir.dt.bfloat16, name="w_sb")
            issue = (
                [("w0", 0), ("x", 0), ("w0", 1), ("x", 1), ("w0", 2), ("w0", 3)]
                + [("x", i) for i in range(2, x_chunks)]
            )
            for kind, i in issue:
                if kind == "w0":
                    nc.sync.dma_start(
                        w_sb0[:, i * kcs : (i + 1) * kcs, :],
                        w_r[:, i * kcs : (i + 1) * kcs, 0:ns],
                    )
                else:
                    nc.sync.dma_start(
                        x_sb[:, :, i * xc : (i + 1) * xc],
                        xt_r[:, :, i * xc : (i + 1) * xc],
                    )

            for _rep in range(repeat):  # repeat>1 only for timing calibration
                for n in range(nt_n):
                    if _rep == 0 and n == 0:
                        w_sb = w_sb0
                    else:
                        w_sb = wpool.tile([P, ko_n, ns], mybir.dt.bfloat16, name="w_sb")
                        nc.sync.dma_start(w_sb[:], w_r[:, :, n * ns : (n + 1) * ns])
                    for m in range(mt_n):
                        ps = pspool.tile([P, ns], mybir.dt.float32)
                        for ko in range(ko_n):
                            nc.tensor.matmul(
                                ps[:],
                                x_sb[:, ko, m * P : (m + 1) * P],
                                w_sb[:, ko, :],
                                start=(ko == 0),
                                stop=(ko == ko_n - 1),
                            )
                        o_sb = opool.tile([P, ns], mybir.dt.float32)
                        nc.vector.tensor_copy(o_sb[:], ps[:])
                        nc.sync.dma_start(
                            out[m * P : (m + 1) * P, n * ns : (n + 1) * ns], o_sb[:]
                        )

    nc.finalize()
    return nc


def _host_prep(x, weight, lora_a, lora_b):
    x = np.asarray(x, dtype=np.float32)
    weight = np.asarray(weight, dtype=np.float32)
    la = np.asarray(lora_a, dtype=np.float32)
    lb = np.asarray(lora_b, dtype=np.float32)

    # Symmetric per-row absmax int8 quant-dequant, matching the reference's
    # fp32 elementwise ops bit-for-bit (max/div/round/clip are exact or
    # correctly rounded in IEEE f32 on any backend).
    abs_max = np.max(np.abs(weight), axis=-1, keepdims=True)
    scale = (abs_max / np.float32(127.0)).astype(np.float32)
    wq = np.clip(
        np.round(weight / (scale + np.float32(1e-8))), -128.0, 127.0
    ).astype(np.float32)
    w_dq = wq * scale

    w_eff = w_dq.T + np.float32(SCALING) * (la @ lb)
    w_bf = w_eff.astype(ml_dtypes.bfloat16)

    x_bf = x.astype(ml_dtypes.bfloat16)
    xt_shards = [
        np.ascontiguousarray(x_bf[c * TPC : (c + 1) * TPC].T) for c in range(N_CORES)
    ]
    return xt_shards, np.ascontiguousarray(w_bf)


def kernel(x, weight, lora_a, lora_b):
    from concourse.bass_utils import run_bass_kernel_spmd

    xt_shards, w_bf = _host_prep(x, weight, lora_a, lora_b)

    if "nc" not in _NC_CACHE:
        _NC_CACHE["nc"] = _build_nc()
    nc = _NC_CACHE["nc"]

    in_maps = [{"xt": xt_shards[c], "w": w_bf} for c in range(N_CORES)]
    res = run_bass_kernel_spmd(nc, in_maps, core_ids=list(range(N_CORES)))
    out = np.concatenate([res.results[c]["out"] for c in range(N_CORES)], axis=0)
    return out

